# revision 15
# baseline (speedup 1.0000x reference)
"""Multi-head attention (16 heads, E=1024, seq=2048, batch=4) on 8 NeuronCores.

Sharding: core = 2*b + g  (b = batch 0..3, g = head-group 0..1, 8 heads each).
Each core computes its batch's QKV for its 8 heads, attention, and a partial
output projection (rows of W_out for its heads); host sums the two partials
per batch and adds b_out.

On-chip layout avoids all transposes:
  - host supplies x^T [1024, 2048] per core
  - q^T,k^T computed as (W^T x^T)  -> [qk_col, seq]   (lhsT = W chunk)
  - v computed naturally as x @ W_v -> [seq, v_col]   (lhsT = x^T chunk)
  - scores^T[sk, sq] = (k^T chunk)^T.T @ q^T  (lhsT = k^T slice, rhs = q^T)
  - softmax denominator via an appended ones-column in the PV lhsT
  - PV: out^T[d(+1), sq] = [v | 1]^T @ attn^T, accumulated over sk chunks
  - proj: y[sq, :] from lhsT = out^T tiles, rhs = W_out rows for this group

All matmuls run in float32r (TF32-like, 1 cyc/row) with fp32 PSUM accumulate.
"""

import sys

sys.path.insert(0, "/opt/trn_rl_repo")

import numpy as np

import concourse.bacc as bacc
import concourse.mybir as mybir
import concourse.tile as tile
from concourse import bass_utils

P = 128
SEQ = 2048
EMB = 1024
N_HEADS_CORE = 8
D_HEAD = 64
QK_COLS = 1024          # q(512) + k(512) for this core's heads
V_COLS = 512
VA = D_HEAD + 1         # v columns per head incl. ones column
N_CORES = 8
NORM = 0.125            # 1/sqrt(64), folded into W_q/b_q on host

F32 = mybir.dt.float32
F32R = mybir.dt.float32r
AF = mybir.ActivationFunctionType

_CACHED = None


def _build():
    nc = bacc.Bacc("TRN2", target_bir_lowering=False, debug=False,
                   enable_asserts=True, num_devices=N_CORES)

    xT = nc.dram_tensor("xT", [EMB, SEQ], F32R, kind="ExternalInput").ap()
    wqk = nc.dram_tensor("wqk", [EMB, QK_COLS], F32R, kind="ExternalInput").ap()
    wv = nc.dram_tensor("wv", [EMB, V_COLS], F32R, kind="ExternalInput").ap()
    wo = nc.dram_tensor("wo", [V_COLS, EMB], F32R, kind="ExternalInput").ap()
    bqk = nc.dram_tensor("bqk", [P, QK_COLS // P], F32, kind="ExternalInput").ap()
    bv = nc.dram_tensor("bv", [1, V_COLS], F32, kind="ExternalInput").ap()
    ones8 = nc.dram_tensor("ones8", [1, N_HEADS_CORE], F32R, kind="ExternalInput").ap()
    out = nc.dram_tensor("out", [SEQ, EMB], F32, kind="ExternalOutput").ap()

    KC = EMB // P          # 8 contraction chunks
    NQK = QK_COLS // P     # 8 qk col tiles (0..3 q, 4..7 k)
    NSQ = SEQ // 512       # 4 seq blocks of 512
    NSC = SEQ // P         # 16 seq chunks of 128
    SQW = 1024             # phase-B sq block width
    NJ = SEQ // SQW        # 2

    with tile.TileContext(nc) as tc:
        with tc.tile_pool(name="persist", bufs=1) as persist:
            # ---- persistent tiles ----
            qT = [persist.tile([P, SEQ], F32R, tag=f"qT{t}", name=f"qT{t}") for t in range(4)]
            kT = [persist.tile([P, SEQ], F32R, tag=f"kT{t}", name=f"kT{t}") for t in range(4)]
            vsb = [persist.tile([P, N_HEADS_CORE * VA], F32R, tag=f"v{s}", name=f"v{s}")
                   for s in range(NSC)]
            bqk_sb = persist.tile([P, QK_COLS // P], F32, tag="bqk")
            bv_sb = persist.tile([P, V_COLS], F32, tag="bv")

            nc.sync.dma_start(bqk_sb[:], bqk)
            nc.sync.dma_start(bv_sb[:], bv[0:1, :].broadcast_to([P, V_COLS]))

            # ---- phase A: QKV projections ----
            with tc.tile_pool(name="phA", bufs=1) as phA, \
                 tc.tile_pool(name="wrot", bufs=2) as wrot, \
                 tc.tile_pool(name="psA", bufs=4, space="PSUM") as psA:
                xT_sb = [phA.tile([P, SEQ], F32R, tag=f"xT{k}", name=f"xTs{k}") for k in range(KC)]
                wv_sb = [phA.tile([P, V_COLS], F32R, tag=f"wv{k}", name=f"wvs{k}")
                         for k in range(KC)]
                for k in range(KC):
                    nc.sync.dma_start(xT_sb[k][:], xT[k * P:(k + 1) * P, :])
                    nc.sync.dma_start(wv_sb[k][:], wv[k * P:(k + 1) * P, :])

                # q^T / k^T : [qk_col, seq]
                for t in range(NQK):
                    wt = []
                    for k in range(KC):
                        w = wrot.tile([P, P], F32R, tag=f"wr{k}", name=f"wr{t}_{k}")
                        nc.sync.dma_start(
                            w[:], wqk[k * P:(k + 1) * P, t * P:(t + 1) * P])
                        wt.append(w)
                    for j in range(NSQ):
                        ps = psA.tile([P, 512], F32, tag="psqk")
                        for k in range(KC):
                            nc.tensor.matmul(
                                ps[:],
                                wt[k][:],
                                xT_sb[k][:, j * 512:(j + 1) * 512],
                                start=(k == 0), stop=(k == KC - 1))
                        dst = qT[t] if t < 4 else kT[t - 4]
                        nc.vector.tensor_scalar_add(
                            dst[:, j * 512:(j + 1) * 512], ps[:],
                            bqk_sb[:, t:t + 1])

                # v natural: [seq, v_col], bias added, ones col appended
                for s in range(NSC):
                    ps = psA.tile([P, V_COLS], F32, tag="psv")
                    for k in range(KC):
                        nc.tensor.matmul(
                            ps[:],
                            xT_sb[k][:, s * P:(s + 1) * P],
                            wv_sb[k][:],
                            start=(k == 0), stop=(k == KC - 1))
                    v3 = vsb[s][:].rearrange("p (h c) -> p h c", c=VA)
                    ps3 = ps[:].rearrange("p (h c) -> p h c", c=D_HEAD)
                    bv3 = bv_sb[:].rearrange("p (h c) -> p h c", c=D_HEAD)
                    nc.vector.tensor_add(v3[:, :, 0:D_HEAD], ps3, bv3)
                    nc.sync.dma_start(
                        v3[:, :, D_HEAD],
                        ones8[0:1, :].broadcast_to([P, N_HEADS_CORE]))

            # ---- phases B+C share the outT / wo pool ----
            with tc.tile_pool(name="bc", bufs=1) as bc:
              outT = [bc.tile([P, SEQ], F32R, tag=f"oT{t}", name=f"oT{t}")
                      for t in range(4)]
              wo_sb = [bc.tile([P, EMB], F32R, tag=f"wo{t}", name=f"wo{t}")
                       for t in range(4)]
              for t in range(4):
                  nc.sync.dma_start(wo_sb[t][:], wo[t * P:(t + 1) * P, :])

              # ---- phase B: attention ----
              with tc.tile_pool(name="attn", bufs=3) as attn_pool, \
                 tc.tile_pool(name="nrm", bufs=2) as nrm_pool, \
                 tc.tile_pool(name="nrmd", bufs=2, space="DRAM") as nrmd_pool, \
                 tc.tile_pool(name="ps_s", bufs=2, space="PSUM") as ps_s_pool, \
                 tc.tile_pool(name="ps_o", bufs=2, space="PSUM") as ps_o_pool:
                for h in range(N_HEADS_CORE):
                    t = h // 2
                    prow = (h % 2) * D_HEAD
                    kTh = kT[t]
                    qTh = qT[t]
                    for j in range(NJ):
                        ps_o = ps_o_pool.tile([VA, SQW], F32, tag="ps_o")
                        for c in range(NSC):
                            ps_s = ps_s_pool.tile([P, SQW], F32, tag="ps_s")
                            for j2 in range(SQW // 512):
                                sq0 = j * SQW + j2 * 512
                                nc.tensor.matmul(
                                    ps_s[:, j2 * 512:(j2 + 1) * 512],
                                    kTh[prow:prow + D_HEAD, c * P:(c + 1) * P],
                                    qTh[prow:prow + D_HEAD, sq0:sq0 + 512],
                                    start=True, stop=True)
                            at = attn_pool.tile([P, SQW], F32R, tag="attnT")
                            nc.scalar.activation(at[:], ps_s[:], AF.Exp)
                            va3 = vsb[c][:].rearrange("p (h c) -> p h c", c=VA)
                            for j2 in range(SQW // 512):
                                nc.tensor.matmul(
                                    ps_o[:, j2 * 512:(j2 + 1) * 512],
                                    va3[:, h, :],
                                    at[:, j2 * 512:(j2 + 1) * 512],
                                    start=(c == 0), stop=(c == NSC - 1))
                        recip = nrm_pool.tile([1, SQW], F32, tag="recip")
                        nc.vector.reciprocal(recip[:], ps_o[D_HEAD:VA, :])
                        recip_d = nrmd_pool.tile([1, SQW], F32, tag="recip_d")
                        nc.sync.dma_start(recip_d[:], recip[:])
                        rbc = nrm_pool.tile([D_HEAD, SQW], F32, tag="rbc")
                        nc.sync.dma_start(
                            rbc[:], recip_d[0:1, :].broadcast_to([D_HEAD, SQW]))
                        nc.vector.tensor_mul(
                            outT[t][prow:prow + D_HEAD, j * SQW:(j + 1) * SQW],
                            ps_o[0:D_HEAD, :], rbc[:])

              # ---- phase C: output projection (partial; host adds pair + b_out)
              with tc.tile_pool(name="osb", bufs=4) as osb_pool, \
                 tc.tile_pool(name="psC", bufs=4, space="PSUM") as psC:
                for s in range(NSC):
                    for y in range(EMB // 512):
                        ps = psC.tile([P, 512], F32, tag="psc")
                        for t in range(4):
                            nc.tensor.matmul(
                                ps[:],
                                outT[t][:, s * P:(s + 1) * P],
                                wo_sb[t][:, y * 512:(y + 1) * 512],
                                start=(t == 0), stop=(t == 3))
                        ot = osb_pool.tile([P, 512], F32, tag="osb")
                        nc.scalar.copy(ot[:], ps[:])
                        nc.sync.dma_start(
                            out[s * P:(s + 1) * P, y * 512:(y + 1) * 512], ot[:])

    nc.compile()
    return nc


def get_nc():
    global _CACHED
    if _CACHED is None:
        _CACHED = _build()
    return _CACHED


def make_in_maps(x, W_qkv, b_qkv, W_out, b_out):
    x = np.asarray(x, dtype=np.float32)
    W_qkv = np.asarray(W_qkv, dtype=np.float32)
    b_qkv = np.asarray(b_qkv, dtype=np.float32)
    W_out = np.asarray(W_out, dtype=np.float32)
    b_out = np.asarray(b_out, dtype=np.float32)

    in_maps = []
    for core in range(N_CORES):
        b, g = divmod(core, 2)
        c0 = g * 512
        wq = W_qkv[:, c0:c0 + 512] * NORM
        wk = W_qkv[:, EMB + c0:EMB + c0 + 512]
        wv_ = W_qkv[:, 2 * EMB + c0:2 * EMB + c0 + 512]
        bq = b_qkv[c0:c0 + 512] * NORM
        bk = b_qkv[EMB + c0:EMB + c0 + 512]
        bv_ = b_qkv[2 * EMB + c0:2 * EMB + c0 + 512]
        in_maps.append({
            "xT": np.ascontiguousarray(x[b].T),
            "wqk": np.ascontiguousarray(np.concatenate([wq, wk], axis=1)),
            "wv": np.ascontiguousarray(wv_),
            "wo": np.ascontiguousarray(W_out[c0:c0 + 512, :]),
            "bqk": np.ascontiguousarray(
                np.concatenate([bq, bk]).reshape(QK_COLS // P, P).T),
            "bv": bv_.reshape(1, V_COLS),
            "ones8": np.ones((1, N_HEADS_CORE), dtype=np.float32),
        })
    return in_maps


def kernel(x, W_qkv, b_qkv, W_out, b_out):
    nc = get_nc()
    b_out = np.asarray(b_out, dtype=np.float32)
    in_maps = make_in_maps(x, W_qkv, b_qkv, W_out, b_out)
    res = bass_utils.run_bass_kernel_spmd(nc, in_maps, core_ids=list(range(N_CORES)))
    outp = np.empty((4, SEQ, EMB), dtype=np.float32)
    for b in range(4):
        outp[b] = res.results[2 * b]["out"] + res.results[2 * b + 1]["out"] + b_out
    return outp


# revision 24
# speedup vs baseline: 1.1734x; 1.1734x over previous
"""Multi-head attention (16 heads, E=1024, seq=2048, batch=4) on 8 NeuronCores.

Sharding: core = 2*b + g  (b = batch 0..3, g = head-group 0..1, 8 heads each).
Each core computes its batch's QKV for its 8 heads, attention, and a partial
output projection (rows of W_out for its heads); host sums the two partials
per batch and adds b_out.

On-chip layout avoids all transposes:
  - host supplies x^T [1024, 2048] per core
  - q^T,k^T computed as (W^T x^T)  -> [qk_col, seq]   (lhsT = W chunk)
  - v computed naturally as x @ W_v -> [seq, v_col]   (lhsT = x^T chunk)
  - scores^T[sk, sq] = (k^T chunk)^T.T @ q^T  (lhsT = k^T slice, rhs = q^T)
  - softmax denominator via an appended ones-column in the PV lhsT
  - PV: out^T[d(+1), sq] = [v | 1]^T @ attn^T, accumulated over sk chunks
  - proj: y[sq, :] from lhsT = out^T tiles, rhs = W_out rows for this group

All matmuls run in float32r (TF32-like, 1 cyc/row) with fp32 PSUM accumulate.
"""

import sys

sys.path.insert(0, "/opt/trn_rl_repo")

import numpy as np

import concourse.bacc as bacc
import concourse.mybir as mybir
import concourse.tile as tile
from concourse import bass_utils

P = 128
SEQ = 2048
EMB = 1024
N_HEADS_CORE = 8
D_HEAD = 64
QK_COLS = 1024          # q(512) + k(512) for this core's heads
V_COLS = 512
VA = D_HEAD + 1         # v columns per head incl. ones column
N_CORES = 8
NORM = 0.125            # 1/sqrt(64), folded into W_q/b_q on host

F32 = mybir.dt.float32
F32R = mybir.dt.float32r
AF = mybir.ActivationFunctionType

_CACHED = None


def _build():
    nc = bacc.Bacc("TRN2", target_bir_lowering=False, debug=False,
                   enable_asserts=True, num_devices=N_CORES)

    xT = nc.dram_tensor("xT", [EMB, SEQ], F32R, kind="ExternalInput").ap()
    wqk = nc.dram_tensor("wqk", [EMB, QK_COLS], F32R, kind="ExternalInput").ap()
    wv = nc.dram_tensor("wv", [EMB, V_COLS], F32R, kind="ExternalInput").ap()
    wo = nc.dram_tensor("wo", [V_COLS, EMB], F32R, kind="ExternalInput").ap()
    bqk = nc.dram_tensor("bqk", [P, QK_COLS // P], F32, kind="ExternalInput").ap()
    bv = nc.dram_tensor("bv", [1, V_COLS], F32, kind="ExternalInput").ap()
    ones8 = nc.dram_tensor("ones8", [1, N_HEADS_CORE], F32R, kind="ExternalInput").ap()
    out = nc.dram_tensor("out", [SEQ, EMB], F32, kind="ExternalOutput").ap()

    KC = EMB // P          # 8 contraction chunks
    NSC = SEQ // P         # 16 seq chunks of 128
    NJB = SEQ // 512       # 4 sq blocks of 512

    with tile.TileContext(nc) as tc:
      with tc.tile_pool(name="persist", bufs=1) as persist, \
           tc.tile_pool(name="qkT", bufs=2) as qkT_pool, \
           tc.tile_pool(name="oTp", bufs=1) as oT_pool, \
           tc.tile_pool(name="attn", bufs=3) as attn_pool, \
           tc.tile_pool(name="nrm", bufs=2) as nrm_pool, \
           tc.tile_pool(name="nrmd", bufs=2, space="DRAM") as nrmd_pool, \
           tc.tile_pool(name="ps_s", bufs=2, space="PSUM") as ps_s_pool, \
           tc.tile_pool(name="ps_o0", bufs=1, space="PSUM") as ps_o0_pool, \
           tc.tile_pool(name="ps_o1", bufs=1, space="PSUM") as ps_o1_pool:
        ps_o_pools = [ps_o0_pool, ps_o1_pool]
        vsb = [persist.tile([P, N_HEADS_CORE * VA], F32R, tag=f"v{s}", name=f"v{s}")
               for s in range(NSC)]
        bqk_sb = persist.tile([P, QK_COLS // P], F32, tag="bqk")
        bv_sb = persist.tile([P, V_COLS], F32, tag="bv")
        nc.sync.dma_start(bqk_sb[:], bqk)
        nc.sync.dma_start(bv_sb[:], bv[0:1, :].broadcast_to([P, V_COLS]))

        qT = {}
        kT = {}
        outT = [oT_pool.tile([P, SEQ], F32R, tag=f"oT{t}", name=f"oT{t}")
                for t in range(4)]

        def emit_B_pair(t, fillers, after_jb=None):
            """Head pair (2t, 2t+1): rows 0-63 / 64-127 of qT[t]/kT[t].
            Per chunk one ps_s [128,1024] = [A sq512 | B sq512]; scores
            row-packed, one exp for both heads, PV splits to per-head
            accumulators. `fillers` are thunks sprinkled into the chunk
            stream to fill PE slack under the ACT-bound exp pipeline."""
            kTh = kT[t]
            qTh = qT[t]
            it = 0
            fi = 0
            nfill = len(fillers)
            for j in range(NJB):
                sq0 = j * 512
                ps_os = [ps_o_pools[hh].tile([VA, 512], F32, tag=f"ps_o{hh}",
                                             name=f"ps_o{t}_{j}_{hh}")
                         for hh in range(2)]

                def scores(c):
                    ps_s = ps_s_pool.tile([P, 2 * 512], F32, tag="ps_s",
                                          name=f"ps_s{t}_{j}_{c}")
                    for hh in range(2):
                        pr = hh * D_HEAD
                        nc.tensor.matmul(
                            ps_s[:, hh * 512:(hh + 1) * 512],
                            kTh[pr:pr + D_HEAD, c * P:(c + 1) * P],
                            qTh[pr:pr + D_HEAD, sq0:sq0 + 512],
                            start=True, stop=True, tile_position=(pr, 0))
                    return ps_s

                ps_s = scores(0)
                for c in range(NSC):
                    at = attn_pool.tile([P, 2 * 512], F32R, tag="attnT",
                                        name=f"at{t}_{j}_{c}")
                    nc.scalar.activation(at[:], ps_s[:], AF.Exp)
                    if c + 1 < NSC:
                        ps_s = scores(c + 1)
                    va3 = vsb[c][:].rearrange("p (h c) -> p h c", c=VA)
                    for hh in range(2):
                        nc.tensor.matmul(
                            ps_os[hh][:],
                            va3[:, 2 * t + hh, :],
                            at[:, hh * 512:(hh + 1) * 512],
                            start=(c == 0), stop=(c == NSC - 1))
                    it += 1
                    if nfill and it % 8 == 4 and fi < nfill:
                        fillers[fi]()
                        fi += 1

                recip_d = nrmd_pool.tile([2, 512], F32, tag="recip_d",
                                         name=f"recip_d{t}_{j}")
                outUs = []
                for hh in range(2):
                    outU = nrm_pool.tile([VA, 512], F32, tag=f"outU{hh}",
                                         name=f"outU{t}_{j}_{hh}", bufs=1)
                    nc.vector.tensor_copy(outU[:], ps_os[hh][:])
                    nc.vector.reciprocal(outU[D_HEAD:VA, :], outU[D_HEAD:VA, :])
                    nc.sync.dma_start(recip_d[hh:hh + 1, :], outU[D_HEAD:VA, :])
                    outUs.append(outU)
                for hh in range(2):
                    rbc = nrm_pool.tile([D_HEAD, 512], F32, tag=f"rbc{hh}",
                                        name=f"rbc{t}_{j}_{hh}", bufs=1)
                    nc.sync.dma_start(
                        rbc[:],
                        recip_d[hh:hh + 1, :].broadcast_to([D_HEAD, 512]))
                    nc.vector.tensor_mul(
                        outT[t][hh * D_HEAD:(hh + 1) * D_HEAD, sq0:sq0 + 512],
                        outUs[hh][0:D_HEAD, :], rbc[:])
                if after_jb is not None and j >= 1:
                    after_jb(j - 1)
            while fi < nfill:
                fillers[fi]()
                fi += 1
            if after_jb is not None:
                after_jb(NJB - 1)

        # ---- phase A scaffolding (xT, wv, rotating weight slices) ----
        with tc.tile_pool(name="xTp", bufs=1) as xTp, \
             tc.tile_pool(name="wvp", bufs=1) as wvp, \
             tc.tile_pool(name="wrot", bufs=2) as wrot, \
             tc.tile_pool(name="psA", bufs=2, space="PSUM") as psA:
            xT_sb = [xTp.tile([P, SEQ], F32R, tag=f"xT{k}", name=f"xTs{k}")
                     for k in range(KC)]
            wv_sb = [wvp.tile([P, V_COLS], F32R, tag=f"wv{k}", name=f"wvs{k}")
                     for k in range(KC)]
            for k in range(KC):
                nc.sync.dma_start(xT_sb[k][:], xT[k * P:(k + 1) * P, :])
            for k in range(KC):
                nc.sync.dma_start(wv_sb[k][:], wv[k * P:(k + 1) * P, :])

            def emit_wr_dma(t):
                wt = []
                for k in range(KC):
                    w = wrot.tile([P, P], F32R, tag=f"wr{k}", name=f"wr{t}_{k}")
                    nc.gpsimd.dma_start(
                        w[:], wqk[k * P:(k + 1) * P, t * P:(t + 1) * P])
                    wt.append(w)
                return wt

            def emit_qk_col(t, wt):
                """One column tile of q^T (t<4) or k^T (t>=4): 4 psum groups."""
                if t < 4:
                    dst = qT[t] = qkT_pool.tile([P, SEQ], F32R, tag="qTa",
                                                name=f"qT{t}")
                else:
                    dst = kT[t - 4] = qkT_pool.tile([P, SEQ], F32R, tag="kTa",
                                                    name=f"kT{t-4}")

                def one_group(j):
                    def go():
                        ps = psA.tile([P, 512], F32, tag="psA",
                                      name=f"psqk{t}_{j}")
                        for k in range(KC):
                            nc.tensor.matmul(
                                ps[:], wt[k][:],
                                xT_sb[k][:, j * 512:(j + 1) * 512],
                                start=(k == 0), stop=(k == KC - 1))
                        nc.vector.tensor_scalar_add(
                            dst[:, j * 512:(j + 1) * 512], ps[:],
                            bqk_sb[:, t:t + 1])
                    return go
                return [one_group(j) for j in range(NJB)]

            def emit_v_group(s):
                ps = psA.tile([P, V_COLS], F32, tag="psA", name=f"psv{s}")
                for k in range(KC):
                    nc.tensor.matmul(
                        ps[:],
                        xT_sb[k][:, s * P:(s + 1) * P],
                        wv_sb[k][:],
                        start=(k == 0), stop=(k == KC - 1))
                v3 = vsb[s][:].rearrange("p (h c) -> p h c", c=VA)
                ps3 = ps[:].rearrange("p (h c) -> p h c", c=D_HEAD)
                bv3 = bv_sb[:].rearrange("p (h c) -> p h c", c=D_HEAD)
                nc.vector.tensor_add(v3[:, :, 0:D_HEAD], ps3, bv3)
                nc.sync.dma_start(
                    v3[:, :, D_HEAD],
                    ones8[0:1, :].broadcast_to([P, N_HEADS_CORE]))

            # head: q^T/k^T for pair 0, then all of v
            for t in (0, 4):
                wt = emit_wr_dma(t)
                for g in emit_qk_col(t, wt):
                    g()
            for s in range(NSC):
                emit_v_group(s)

            # B pairs 0-2, with pair p+1's q^T/k^T production as filler
            for pair in range(3):
                fillers = []
                for t in (pair + 1, pair + 5):
                    wt = emit_wr_dma(t)
                    fillers.extend(emit_qk_col(t, wt))
                emit_B_pair(pair, fillers)

        # ---- pair 3 + projection (xT/wv freed; wo loads into that space)
        with tc.tile_pool(name="wop", bufs=1) as wop, \
             tc.tile_pool(name="osb", bufs=2) as osb_pool, \
             tc.tile_pool(name="psC", bufs=2, space="PSUM") as psC:
            wo_sb = [wop.tile([P, EMB], F32R, tag=f"wo{t}", name=f"wo{t}")
                     for t in range(4)]
            for t in range(4):
                nc.sync.dma_start(wo_sb[t][:], wo[t * P:(t + 1) * P, :])

            def emit_C_jb(jb):
                for s in range(4 * jb, 4 * jb + 4):
                    for y in range(EMB // 512):
                        ps = psC.tile([P, 512], F32, tag="psc",
                                      name=f"psc{s}_{y}")
                        for t in range(4):
                            nc.tensor.matmul(
                                ps[:],
                                outT[t][:, s * P:(s + 1) * P],
                                wo_sb[t][:, y * 512:(y + 1) * 512],
                                start=(t == 0), stop=(t == 3))
                        ot = osb_pool.tile([P, 512], F32, tag="osb",
                                           name=f"osb{s}_{y}")
                        nc.vector.tensor_copy(ot[:], ps[:])
                        nc.sync.dma_start(
                            out[s * P:(s + 1) * P, y * 512:(y + 1) * 512],
                            ot[:])

            emit_B_pair(3, [], after_jb=emit_C_jb)

    nc.compile()
    return nc


def get_nc():
    global _CACHED
    if _CACHED is None:
        _CACHED = _build()
    return _CACHED


def make_in_maps(x, W_qkv, b_qkv, W_out, b_out):
    x = np.asarray(x, dtype=np.float32)
    W_qkv = np.asarray(W_qkv, dtype=np.float32)
    b_qkv = np.asarray(b_qkv, dtype=np.float32)
    W_out = np.asarray(W_out, dtype=np.float32)
    b_out = np.asarray(b_out, dtype=np.float32)

    in_maps = []
    for core in range(N_CORES):
        b, g = divmod(core, 2)
        c0 = g * 512
        wq = W_qkv[:, c0:c0 + 512] * NORM
        wk = W_qkv[:, EMB + c0:EMB + c0 + 512]
        wv_ = W_qkv[:, 2 * EMB + c0:2 * EMB + c0 + 512]
        bq = b_qkv[c0:c0 + 512] * NORM
        bk = b_qkv[EMB + c0:EMB + c0 + 512]
        bv_ = b_qkv[2 * EMB + c0:2 * EMB + c0 + 512]
        in_maps.append({
            "xT": np.ascontiguousarray(x[b].T),
            "wqk": np.ascontiguousarray(np.concatenate([wq, wk], axis=1)),
            "wv": np.ascontiguousarray(wv_),
            "wo": np.ascontiguousarray(W_out[c0:c0 + 512, :]),
            "bqk": np.ascontiguousarray(
                np.concatenate([bq, bk]).reshape(QK_COLS // P, P).T),
            "bv": bv_.reshape(1, V_COLS),
            "ones8": np.ones((1, N_HEADS_CORE), dtype=np.float32),
        })
    return in_maps


def kernel(x, W_qkv, b_qkv, W_out, b_out):
    nc = get_nc()
    b_out = np.asarray(b_out, dtype=np.float32)
    in_maps = make_in_maps(x, W_qkv, b_qkv, W_out, b_out)
    res = bass_utils.run_bass_kernel_spmd(nc, in_maps, core_ids=list(range(N_CORES)))
    outp = np.empty((4, SEQ, EMB), dtype=np.float32)
    for b in range(4):
        outp[b] = res.results[2 * b]["out"] + res.results[2 * b + 1]["out"] + b_out
    return outp


# revision 35
# speedup vs baseline: 1.4565x; 1.2412x over previous
"""Multi-head attention (16 heads, E=1024, seq=2048, batch=4) on 8 NeuronCores.

Sharding: core = 2*b + g  (b = batch 0..3, g = head-group 0..1, 8 heads each).
Each core computes its batch's QKV for its 8 heads, attention, and a partial
output projection (rows of W_out for its heads); host sums the two partials
per batch and adds b_out.

On-chip layout avoids all transposes:
  - host supplies x^T [1024, 2048] per core
  - q^T,k^T computed as (W^T x^T)  -> [qk_col, seq]   (lhsT = W chunk)
  - v computed naturally as x @ W_v -> [seq, v_col]   (lhsT = x^T chunk)
  - scores^T[sk, sq] = (k^T chunk)^T.T @ q^T  (lhsT = k^T slice, rhs = q^T)
  - softmax denominator via an appended ones-column in the PV lhsT
  - PV: out^T[d(+1), sq] = [v | 1]^T @ attn^T, accumulated over sk chunks
  - proj: y[sq, :] from lhsT = out^T tiles, rhs = W_out rows for this group

All matmuls run in float32r (TF32-like, 1 cyc/row) with fp32 PSUM accumulate.
"""

import sys

sys.path.insert(0, "/opt/trn_rl_repo")

import numpy as np

import concourse.bacc as bacc
import concourse.mybir as mybir
import concourse.tile as tile
from concourse import bass_utils

P = 128
SEQ = 2048
EMB = 1024
N_HEADS_CORE = 8
D_HEAD = 64
QK_COLS = 1024          # q(512) + k(512) for this core's heads
V_COLS = 512
VA = D_HEAD + 1         # v columns per head incl. ones column
N_CORES = 8
NORM = 0.125            # 1/sqrt(64), folded into W_q/b_q on host

F32 = mybir.dt.float32
F32R = mybir.dt.float32r
AF = mybir.ActivationFunctionType

_CACHED = None


def _build():
    nc = bacc.Bacc("TRN2", target_bir_lowering=False, debug=False,
                   enable_asserts=True, num_devices=N_CORES)

    xT = nc.dram_tensor("xT", [EMB, SEQ], F32R, kind="ExternalInput").ap()
    wqk = nc.dram_tensor("wqk", [EMB, QK_COLS], F32R, kind="ExternalInput").ap()
    wv = nc.dram_tensor("wv", [EMB, V_COLS], F32R, kind="ExternalInput").ap()
    wo = nc.dram_tensor("wo", [V_COLS, EMB], F32R, kind="ExternalInput").ap()
    bqk = nc.dram_tensor("bqk", [P, QK_COLS // P], F32, kind="ExternalInput").ap()
    bv = nc.dram_tensor("bv", [1, V_COLS], F32, kind="ExternalInput").ap()
    out = nc.dram_tensor("out", [SEQ, EMB], F32, kind="ExternalOutput").ap()

    KC = EMB // P          # 8 contraction chunks
    NSC = SEQ // P         # 16 seq chunks of 128
    NJB = SEQ // 512       # 4 sq blocks of 512

    with tile.TileContext(nc) as tc:
      with tc.tile_pool(name="persist", bufs=1) as persist, \
           tc.tile_pool(name="qkT", bufs=2) as qkT_pool, \
           tc.tile_pool(name="oTp", bufs=1) as oT_pool, \
           tc.tile_pool(name="attn", bufs=2) as attn_pool, \
           tc.tile_pool(name="nrm", bufs=2) as nrm_pool, \
           tc.tile_pool(name="ps_s", bufs=2, space="PSUM") as ps_s_pool, \
           tc.tile_pool(name="ps_o0", bufs=1, space="PSUM") as ps_o0_pool, \
           tc.tile_pool(name="ps_o1", bufs=1, space="PSUM") as ps_o1_pool:
        ps_o_pools = [ps_o0_pool, ps_o1_pool]
        vsb = [persist.tile([P, N_HEADS_CORE * VA], F32R, tag=f"v{s}", name=f"v{s}")
               for s in range(NSC)]
        bqk_sb = persist.tile([P, QK_COLS // P], F32, tag="bqk")
        bv_sb = persist.tile([P, V_COLS], F32, tag="bv")
        nc.sync.dma_start(bqk_sb[:], bqk)
        nc.sync.dma_start(bv_sb[:], bv[0:1, :].broadcast_to([P, V_COLS]))
        ones_sb = persist.tile([P, D_HEAD], F32R, tag="ones")
        nc.vector.tensor_scalar(ones_sb[:], bv_sb[:, 0:D_HEAD], 0.0, 1.0,
                                mybir.AluOpType.mult, mybir.AluOpType.add)

        qT = {}
        kT = {}
        outT = [oT_pool.tile([P, SEQ], F32R, tag=f"oT{t}", name=f"oT{t}")
                for t in range(4)]

        pending = [None]

        def emit_B_pair(t, fillers, scratch_pool, after_jb=None, dynamic=False,
                        flush=False):
            """Head pair (2t, 2t+1): rows 0-63 / 64-127 of qT[t]/kT[t].
            Per chunk one ps_s [128,1024] = [A sq512 | B sq512]; scores
            row-packed, one exp for both heads, PV splits to per-head
            accumulators. `fillers` are thunks sprinkled into the chunk
            stream to fill PE slack under the ACT-bound exp pipeline."""
            kTh = kT[t]
            qTh = qT[t]
            it = 0
            fi = 0
            nfill = len(fillers)
            for j in range(NJB):
                sq0 = j * 512
                ps_os = [ps_o_pools[hh].tile([VA, 512], F32, tag=f"ps_o{hh}",
                                             name=f"ps_o{t}_{j}_{hh}")
                         for hh in range(2)]

                def scores(c):
                    ps_s = ps_s_pool.tile([P, 2 * 512], F32, tag="ps_s",
                                          name=f"ps_s{t}_{j}_{c}")
                    for hh in range(2):
                        pr = hh * D_HEAD
                        nc.tensor.matmul(
                            ps_s[:, hh * 512:(hh + 1) * 512],
                            kTh[pr:pr + D_HEAD, c * P:(c + 1) * P],
                            qTh[pr:pr + D_HEAD, sq0:sq0 + 512],
                            start=True, stop=True, tile_position=(pr, 0))
                    return ps_s

                ps_s = scores(0)
                for c in range(NSC):
                    at = attn_pool.tile([P, 2 * 512], F32R, tag="attnT",
                                        name=f"at{t}_{j}_{c}")
                    nc.scalar.activation(at[:], ps_s[:], AF.Exp)
                    if c + 1 < NSC:
                        ps_s = scores(c + 1)
                    va3 = vsb[c][:].rearrange("p (h c) -> p h c", c=VA)
                    for hh in range(2):
                        nc.tensor.matmul(
                            ps_os[hh][:],
                            va3[:, 2 * t + hh, :],
                            at[:, hh * 512:(hh + 1) * 512],
                            start=(c == 0), stop=(c == NSC - 1))
                    it += 1
                    if c == 10 and pending[0] is not None:
                        fin = pending[0]
                        pending[0] = None
                        fin()
                    if dynamic:
                        budget = 3
                        while fi < len(fillers) and budget > 0:
                            fillers[fi]()
                            fi += 1
                            budget -= 1
                    else:
                        while nfill and fi < (nfill * it) // 64 and fi < nfill:
                            fillers[fi]()
                            fi += 1

                # stage 1 (DVE): evacuate ps_o, reciprocal of denominators
                outUs = []
                rcasts = []
                for hh in range(2):
                    outU = nrm_pool.tile([VA, 512], F32, tag=f"outU{hh}",
                                         name=f"outU{t}_{j}_{hh}", bufs=1)
                    nc.vector.tensor_copy(outU[:], ps_os[hh][:])
                    rcast = nrm_pool.tile([VA, 512], F32R, tag=f"rcast{hh}",
                                          name=f"rcast{t}_{j}_{hh}", bufs=1)
                    with nc.allow_low_precision(reason="f32r recip for PE bcast"):
                        nc.vector.reciprocal(rcast[D_HEAD:VA, :],
                                             outU[D_HEAD:VA, :])
                    outUs.append(outU)
                    rcasts.append(rcast)

                # stage 2 (PE bcast + DVE mul): deferred into the next
                # j-block's chunk stream so the PE never waits on the
                # reciprocal chain
                def make_fin(jj, sq00, oUs, rcs):
                    def fin():
                        for hh in range(2):
                            psb = scratch_pool.tile(
                                [P, 512], F32, tag=scratch_pool.name + "_t",
                                name=f"psb{t}_{jj}_{hh}")
                            nc.tensor.matmul(psb[0:D_HEAD, :],
                                             ones_sb[D_HEAD:D_HEAD + 1, :],
                                             rcs[hh][D_HEAD:VA, :],
                                             start=True, stop=True,
                                             tile_position=(D_HEAD, 0))
                            nc.vector.tensor_mul(
                                outT[t][hh * D_HEAD:(hh + 1) * D_HEAD,
                                        sq00:sq00 + 512],
                                oUs[hh][0:D_HEAD, :], psb[0:D_HEAD, :])
                        if after_jb is not None:
                            fillers.extend(after_jb(jj))
                    return fin

                pending[0] = make_fin(j, sq0, outUs, rcasts)
            if flush and pending[0] is not None:
                fin = pending[0]
                pending[0] = None
                fin()
            while fi < len(fillers):
                fillers[fi]()
                fi += 1

        # ---- phase A scaffolding (xT, wv, rotating weight slices) ----
        with tc.tile_pool(name="xTp", bufs=1) as xTp, \
             tc.tile_pool(name="wvp", bufs=1) as wvp, \
             tc.tile_pool(name="wrot", bufs=3) as wrot, \
             tc.tile_pool(name="psA", bufs=2, space="PSUM") as psA:
            xT_sb = [xTp.tile([P, SEQ], F32R, tag=f"xT{k}", name=f"xTs{k}")
                     for k in range(KC)]
            wv_sb = [wvp.tile([P, V_COLS], F32R, tag=f"wv{k}", name=f"wvs{k}")
                     for k in range(KC)]
            for k in range(KC):
                nc.sync.dma_start(xT_sb[k][:], xT[k * P:(k + 1) * P, :])
            for k in range(KC):
                nc.sync.dma_start(wv_sb[k][:], wv[k * P:(k + 1) * P, :])

            def emit_wr_dma(t):
                wt = []
                for k in range(KC):
                    w = wrot.tile([P, P], F32R, tag=f"wr{k}", name=f"wr{t}_{k}")
                    nc.gpsimd.dma_start(
                        w[:], wqk[k * P:(k + 1) * P, t * P:(t + 1) * P])
                    wt.append(w)
                return wt

            def emit_qk_col(t, wt, fine=False):
                """One column tile of q^T (t<4) or k^T (t>=4): 4 psum groups.
                fine=True returns one thunk per matmul (32 thunks) so a
                group can be dribbled into B's per-chunk PE slack."""
                if t < 4:
                    dst = qT[t] = qkT_pool.tile([P, SEQ], F32R, tag="qTa",
                                                name=f"qT{t}")
                else:
                    dst = kT[t - 4] = qkT_pool.tile([P, SEQ], F32R, tag="kTa",
                                                    name=f"kT{t-4}")

                state = {}

                def one_mm(j, k):
                    def go():
                        if k == 0:
                            state[j] = psA.tile([P, 512], F32, tag="psA_t",
                                                name=f"psqk{t}_{j}")
                        ps = state[j]
                        nc.tensor.matmul(
                            ps[:], wt[k][:],
                            xT_sb[k][:, j * 512:(j + 1) * 512],
                            start=(k == 0), stop=(k == KC - 1))
                        if k == KC - 1:
                            nc.vector.tensor_scalar_add(
                                dst[:, j * 512:(j + 1) * 512], ps[:],
                                bqk_sb[:, t:t + 1])
                    return go

                def one_group(j):
                    def go():
                        for k in range(KC):
                            one_mm(j, k)()
                    return go

                if fine:
                    return [one_mm(j, k) for j in range(NJB) for k in range(KC)]
                return [one_group(j) for j in range(NJB)]

            def emit_v_group(s):
                ps = psA.tile([P, V_COLS], F32, tag="psA_t", name=f"psv{s}")
                for k in range(KC):
                    nc.tensor.matmul(
                        ps[:],
                        xT_sb[k][:, s * P:(s + 1) * P],
                        wv_sb[k][:],
                        start=(k == 0), stop=(k == KC - 1))
                v3 = vsb[s][:].rearrange("p (h c) -> p h c", c=VA)
                ps3 = ps[:].rearrange("p (h c) -> p h c", c=D_HEAD)
                bv3 = bv_sb[:].rearrange("p (h c) -> p h c", c=D_HEAD)
                nc.vector.tensor_add(v3[:, :, 0:D_HEAD], ps3, bv3)
                nc.vector.tensor_scalar(
                    v3[:, :, D_HEAD], bv_sb[:, 0:N_HEADS_CORE], 0.0, 1.0,
                    mybir.AluOpType.mult, mybir.AluOpType.add)

            # head: q^T/k^T for pair 0, then all of v
            wt01 = {}
            for t in (0, 4):
                wt = emit_wr_dma(t)
                for g in emit_qk_col(t, wt):
                    g()
            wt01[1] = emit_wr_dma(1)   # prefetch pair0's first filler weights
            for s in range(NSC):
                emit_v_group(s)

            # B pairs 0-2, with pair p+1's q^T/k^T production dribbled
            # into the chunk stream one matmul at a time; weight slices
            # prefetched one pair ahead
            for pair in range(3):
                t_lo, t_hi = pair + 1, pair + 5
                wt_lo = wt01.pop(t_lo, None) or emit_wr_dma(t_lo)
                fillers = list(emit_qk_col(t_lo, wt_lo, fine=True))
                wt_hi = emit_wr_dma(t_hi)
                if pair < 2:
                    wt01[pair + 2] = emit_wr_dma(pair + 2)
                fillers.extend(emit_qk_col(t_hi, wt_hi, fine=True))
                emit_B_pair(pair, fillers, psA)
            if pending[0] is not None:
                fin = pending[0]
                pending[0] = None
                fin()

        # ---- pair 3 + projection (xT/wv freed; wo loads into that space)
        with tc.tile_pool(name="wop", bufs=1) as wop, \
             tc.tile_pool(name="osb", bufs=2) as osb_pool, \
             tc.tile_pool(name="psC", bufs=2, space="PSUM") as psC:
            wo_sb = [wop.tile([P, EMB], F32R, tag=f"wo{t}", name=f"wo{t}")
                     for t in range(4)]
            for t in range(4):
                nc.sync.dma_start(wo_sb[t][:], wo[t * P:(t + 1) * P, :])

            cstate = {}

            def one_c_mm(s, y, t):
                def go():
                    if t == 0:
                        cstate[(s, y)] = psC.tile([P, 512], F32, tag="psC_t",
                                                  name=f"psc{s}_{y}")
                    ps = cstate[(s, y)]
                    nc.tensor.matmul(
                        ps[:],
                        outT[t][:, s * P:(s + 1) * P],
                        wo_sb[t][:, y * 512:(y + 1) * 512],
                        start=(t == 0), stop=(t == 3))
                    if t == 3:
                        ot = osb_pool.tile([P, 512], F32, tag="osb",
                                           name=f"osb{s}_{y}")
                        nc.vector.tensor_copy(ot[:], ps[:])
                        nc.sync.dma_start(
                            out[s * P:(s + 1) * P, y * 512:(y + 1) * 512],
                            ot[:])
                return go

            def emit_C_jb(jb):
                return [one_c_mm(s, y, t)
                        for s in range(4 * jb, 4 * jb + 4)
                        for y in range(EMB // 512)
                        for t in range(4)]

            emit_B_pair(3, [], psC, after_jb=emit_C_jb, dynamic=True, flush=True)

    nc.compile()
    return nc


def get_nc():
    global _CACHED
    if _CACHED is None:
        _CACHED = _build()
    return _CACHED


def make_in_maps(x, W_qkv, b_qkv, W_out, b_out):
    x = np.asarray(x, dtype=np.float32)
    W_qkv = np.asarray(W_qkv, dtype=np.float32)
    b_qkv = np.asarray(b_qkv, dtype=np.float32)
    W_out = np.asarray(W_out, dtype=np.float32)
    b_out = np.asarray(b_out, dtype=np.float32)

    in_maps = []
    for core in range(N_CORES):
        b, g = divmod(core, 2)
        c0 = g * 512
        wq = W_qkv[:, c0:c0 + 512] * NORM
        wk = W_qkv[:, EMB + c0:EMB + c0 + 512]
        wv_ = W_qkv[:, 2 * EMB + c0:2 * EMB + c0 + 512]
        bq = b_qkv[c0:c0 + 512] * NORM
        bk = b_qkv[EMB + c0:EMB + c0 + 512]
        bv_ = b_qkv[2 * EMB + c0:2 * EMB + c0 + 512]
        in_maps.append({
            "xT": np.ascontiguousarray(x[b].T),
            "wqk": np.ascontiguousarray(np.concatenate([wq, wk], axis=1)),
            "wv": np.ascontiguousarray(wv_),
            "wo": np.ascontiguousarray(W_out[c0:c0 + 512, :]),
            "bqk": np.ascontiguousarray(
                np.concatenate([bq, bk]).reshape(QK_COLS // P, P).T),
            "bv": bv_.reshape(1, V_COLS),
        })
    return in_maps


def kernel(x, W_qkv, b_qkv, W_out, b_out):
    nc = get_nc()
    b_out = np.asarray(b_out, dtype=np.float32)
    in_maps = make_in_maps(x, W_qkv, b_qkv, W_out, b_out)
    res = bass_utils.run_bass_kernel_spmd(nc, in_maps, core_ids=list(range(N_CORES)))
    outp = np.empty((4, SEQ, EMB), dtype=np.float32)
    for b in range(4):
        outp[b] = res.results[2 * b]["out"] + res.results[2 * b + 1]["out"] + b_out
    return outp


# revision 42
# speedup vs baseline: 1.6484x; 1.1318x over previous
"""Multi-head attention (16 heads, E=1024, seq=2048, batch=4) on 8 NeuronCores.

Sharding: core = 2*b + g  (b = batch 0..3, g = head-group 0..1, 8 heads each).
Each core computes its batch's QKV for its 8 heads, attention, and a partial
output projection (rows of W_out for its heads); host sums the two partials
per batch and adds b_out.

On-chip layout avoids all transposes:
  - host supplies x^T [1024, 2048] per core
  - q^T,k^T computed as (W^T x^T)  -> [qk_col, seq]   (lhsT = W chunk)
  - v computed naturally as x @ W_v -> [seq, v_col]   (lhsT = x^T chunk)
  - scores^T[sk, sq] = (k^T chunk)^T.T @ q^T  (lhsT = k^T slice, rhs = q^T)
  - softmax denominator via an appended ones-column in the PV lhsT
  - PV: out^T[d(+1), sq] = [v | 1]^T @ attn^T, accumulated over sk chunks
  - proj: y[sq, :] from lhsT = out^T tiles, rhs = W_out rows for this group

All matmuls run in float32r (TF32-like, 1 cyc/row) with fp32 PSUM accumulate.
"""

import sys

sys.path.insert(0, "/opt/trn_rl_repo")

import numpy as np

import concourse.bacc as bacc
import concourse.mybir as mybir
import concourse.tile as tile
from concourse import bass_utils

P = 128
SEQ = 2048
EMB = 1024
N_HEADS_CORE = 8
D_HEAD = 64
QK_COLS = 1024          # q(512) + k(512) for this core's heads
V_COLS = 512
VA = D_HEAD + 1         # v columns per head incl. ones column
N_CORES = 8
NORM = 0.125            # 1/sqrt(64), folded into W_q/b_q on host

F32 = mybir.dt.float32
F32R = mybir.dt.float32r
AF = mybir.ActivationFunctionType

_CACHED = None


def _build():
    nc = bacc.Bacc("TRN2", target_bir_lowering=False, debug=False,
                   enable_asserts=True, num_devices=N_CORES)

    xT = nc.dram_tensor("xT", [EMB, SEQ], F32R, kind="ExternalInput").ap()
    wqk = nc.dram_tensor("wqk", [EMB, QK_COLS], F32R, kind="ExternalInput").ap()
    wv = nc.dram_tensor("wv", [EMB, V_COLS], F32R, kind="ExternalInput").ap()
    wo = nc.dram_tensor("wo", [V_COLS, EMB], F32R, kind="ExternalInput").ap()
    bqk = nc.dram_tensor("bqk", [P, QK_COLS // P], F32, kind="ExternalInput").ap()
    bv = nc.dram_tensor("bv", [1, V_COLS], F32, kind="ExternalInput").ap()
    out = nc.dram_tensor("out", [SEQ, EMB], F32, kind="ExternalOutput").ap()

    KC = EMB // P          # 8 contraction chunks
    NSC = SEQ // P         # 16 seq chunks of 128
    NJB = SEQ // 512       # 4 sq blocks of 512

    with tile.TileContext(nc) as tc:
      with tc.tile_pool(name="persist", bufs=1) as persist, \
           tc.tile_pool(name="qkT", bufs=2) as qkT_pool, \
           tc.tile_pool(name="oTp", bufs=1) as oT_pool, \
           tc.tile_pool(name="attn", bufs=2) as attn_pool, \
           tc.tile_pool(name="nrm", bufs=2) as nrm_pool, \
           tc.tile_pool(name="ps_s", bufs=2, space="PSUM") as ps_s_pool, \
           tc.tile_pool(name="ps_o0", bufs=1, space="PSUM") as ps_o0_pool, \
           tc.tile_pool(name="ps_o1", bufs=1, space="PSUM") as ps_o1_pool:
        ps_o_pools = [ps_o0_pool, ps_o1_pool]
        vsb = [persist.tile([P, N_HEADS_CORE * VA], F32R, tag=f"v{s}", name=f"v{s}")
               for s in range(NSC)]
        bqk_sb = persist.tile([P, QK_COLS // P], F32, tag="bqk")
        bv_sb = persist.tile([P, V_COLS], F32, tag="bv")
        nc.sync.dma_start(bqk_sb[:], bqk)
        nc.sync.dma_start(bv_sb[:], bv[0:1, :].broadcast_to([P, V_COLS]))
        ones_sb = persist.tile([P, D_HEAD], F32R, tag="ones")
        nc.vector.tensor_scalar(ones_sb[:], bv_sb[:, 0:D_HEAD], 0.0, 1.0,
                                mybir.AluOpType.mult, mybir.AluOpType.add)

        qT = {}
        kT = {}
        outT = [oT_pool.tile([P, SEQ], F32R, tag=f"oT{t}", name=f"oT{t}")
                for t in range(4)]

        pending = [None]

        def emit_B_pair(t, fillers, scratch_pool, after_jb=None, dynamic=False,
                        flush=False):
            """Head pair (2t, 2t+1): rows 0-63 / 64-127 of qT[t]/kT[t].
            Per chunk one ps_s [128,1024] = [A sq512 | B sq512]; scores
            row-packed, one exp for both heads, PV splits to per-head
            accumulators. `fillers` are thunks sprinkled into the chunk
            stream to fill PE slack under the ACT-bound exp pipeline."""
            kTh = kT[t]
            qTh = qT[t]
            it = 0
            fi = 0
            nfill = len(fillers)
            for j in range(NJB):
                sq0 = j * 512
                ps_os = [ps_o_pools[hh].tile([VA, 512], F32, tag=f"ps_o{hh}",
                                             name=f"ps_o{t}_{j}_{hh}")
                         for hh in range(2)]

                def scores(c):
                    ps_s = ps_s_pool.tile([P, 2 * 512], F32, tag="ps_s",
                                          name=f"ps_s{t}_{j}_{c}")
                    for hh in range(2):
                        pr = hh * D_HEAD
                        nc.tensor.matmul(
                            ps_s[:, hh * 512:(hh + 1) * 512],
                            kTh[pr:pr + D_HEAD, c * P:(c + 1) * P],
                            qTh[pr:pr + D_HEAD, sq0:sq0 + 512],
                            start=True, stop=True, tile_position=(pr, 0))
                    return ps_s

                ps_s = scores(0)
                for c in range(NSC):
                    at = attn_pool.tile([P, 2 * 512], F32R, tag="attnT",
                                        name=f"at{t}_{j}_{c}")
                    nc.scalar.activation(at[:], ps_s[:], AF.Exp)
                    if c + 1 < NSC:
                        ps_s = scores(c + 1)
                    va3 = vsb[c][:].rearrange("p (h c) -> p h c", c=VA)
                    for hh in range(2):
                        nc.tensor.matmul(
                            ps_os[hh][:],
                            va3[:, 2 * t + hh, :],
                            at[:, hh * 512:(hh + 1) * 512],
                            start=(c == 0), stop=(c == NSC - 1))
                    it += 1
                    if c == 10 and pending[0] is not None:
                        fin = pending[0]
                        pending[0] = None
                        fin()
                    if dynamic:
                        budget = 3
                        while fi < len(fillers) and budget > 0:
                            fillers[fi]()
                            fi += 1
                            budget -= 1
                    else:
                        while nfill and fi < (nfill * it) // 64 and fi < nfill:
                            fillers[fi]()
                            fi += 1

                # stage 1 (DVE): evacuate ps_o, reciprocal of denominators
                outUs = []
                rcasts = []
                for hh in range(2):
                    outU = nrm_pool.tile([VA, 512], F32, tag=f"outU{hh}",
                                         name=f"outU{t}_{j}_{hh}", bufs=1)
                    nc.vector.tensor_copy(outU[:], ps_os[hh][:])
                    rcast = nrm_pool.tile([VA, 512], F32R, tag=f"rcast{hh}",
                                          name=f"rcast{t}_{j}_{hh}", bufs=1)
                    with nc.allow_low_precision(reason="denom cast to f32r"):
                        nc.vector.tensor_copy(rcast[D_HEAD:VA, :],
                                              outU[D_HEAD:VA, :])
                    outUs.append(outU)
                    rcasts.append(rcast)

                # stage 2 (PE bcast + DVE mul): deferred into the next
                # j-block's chunk stream so the PE never waits on the
                # reciprocal chain
                def make_fin(jj, sq00, oUs, rcs):
                    def fin():
                        for hh in range(2):
                            psb = scratch_pool.tile(
                                [P, 512], F32, tag=scratch_pool.name + "_t",
                                name=f"psb{t}_{jj}_{hh}")
                            nc.tensor.matmul(psb[0:D_HEAD, :],
                                             ones_sb[D_HEAD:D_HEAD + 1, :],
                                             rcs[hh][D_HEAD:VA, :],
                                             start=True, stop=True,
                                             tile_position=(D_HEAD, 0))
                            rb = nrm_pool.tile([D_HEAD, 512], F32,
                                               tag=f"rb{hh}",
                                               name=f"rb{t}_{jj}_{hh}", bufs=1)
                            nc.vector.reciprocal(rb[:], psb[0:D_HEAD, :])
                            nc.vector.tensor_mul(
                                outT[t][hh * D_HEAD:(hh + 1) * D_HEAD,
                                        sq00:sq00 + 512],
                                oUs[hh][0:D_HEAD, :], rb[:])
                        if after_jb is not None:
                            fillers.extend(after_jb(jj))
                    return fin

                pending[0] = make_fin(j, sq0, outUs, rcasts)
            if pending[0] is not None:
                fin = pending[0]
                pending[0] = None
                fin()
            while fi < len(fillers):
                fillers[fi]()
                fi += 1

        # ---- phase A scaffolding (xT, wv, rotating weight slices) ----
        with tc.tile_pool(name="xTp", bufs=1) as xTp, \
             tc.tile_pool(name="wvp", bufs=1) as wvp, \
             tc.tile_pool(name="wrot", bufs=2) as wrot, \
             tc.tile_pool(name="psA", bufs=2, space="PSUM") as psA:
            xT_sb = [xTp.tile([P, SEQ], F32R, tag=f"xT{k}", name=f"xTs{k}")
                     for k in range(KC)]
            wv_sb = [wvp.tile([P, V_COLS], F32R, tag=f"wv{k}", name=f"wvs{k}")
                     for k in range(KC)]
            for k in range(KC):
                nc.sync.dma_start(xT_sb[k][:], xT[k * P:(k + 1) * P, :])
            for k in range(KC):
                nc.sync.dma_start(wv_sb[k][:], wv[k * P:(k + 1) * P, :])

            def emit_wr_dma(t):
                wt = []
                for k in range(KC):
                    w = wrot.tile([P, P], F32R, tag=f"wr{k}", name=f"wr{t}_{k}")
                    nc.gpsimd.dma_start(
                        w[:], wqk[k * P:(k + 1) * P, t * P:(t + 1) * P])
                    wt.append(w)
                return wt

            def emit_qk_col(t, wt, fine=False):
                """One column tile of q^T (t<4) or k^T (t>=4): 4 psum groups.
                fine=True returns one thunk per matmul (32 thunks) so a
                group can be dribbled into B's per-chunk PE slack."""
                if t < 4:
                    dst = qT[t] = qkT_pool.tile([P, SEQ], F32R, tag="qTa",
                                                name=f"qT{t}")
                else:
                    dst = kT[t - 4] = qkT_pool.tile([P, SEQ], F32R, tag="kTa",
                                                    name=f"kT{t-4}")

                state = {}

                def one_mm(j, k):
                    def go():
                        if k == 0:
                            state[j] = psA.tile([P, 512], F32, tag="psA_t",
                                                name=f"psqk{t}_{j}")
                        ps = state[j]
                        nc.tensor.matmul(
                            ps[:], wt[k][:],
                            xT_sb[k][:, j * 512:(j + 1) * 512],
                            start=(k == 0), stop=(k == KC - 1))
                        if k == KC - 1:
                            nc.vector.tensor_scalar_add(
                                dst[:, j * 512:(j + 1) * 512], ps[:],
                                bqk_sb[:, t:t + 1])
                    return go

                def one_group(j):
                    def go():
                        for k in range(KC):
                            one_mm(j, k)()
                    return go

                if fine:
                    return [one_mm(j, k) for j in range(NJB) for k in range(KC)]
                return [one_group(j) for j in range(NJB)]

            def emit_v_group(s):
                ps = psA.tile([P, V_COLS], F32, tag="psA_t", name=f"psv{s}")
                for k in range(KC):
                    nc.tensor.matmul(
                        ps[:],
                        xT_sb[k][:, s * P:(s + 1) * P],
                        wv_sb[k][:],
                        start=(k == 0), stop=(k == KC - 1))
                v3 = vsb[s][:].rearrange("p (h c) -> p h c", c=VA)
                ps3 = ps[:].rearrange("p (h c) -> p h c", c=D_HEAD)
                bv3 = bv_sb[:].rearrange("p (h c) -> p h c", c=D_HEAD)
                nc.vector.tensor_add(v3[:, :, 0:D_HEAD], ps3, bv3)
                nc.vector.tensor_scalar(
                    v3[:, :, D_HEAD], bv_sb[:, 0:N_HEADS_CORE], 0.0, 1.0,
                    mybir.AluOpType.mult, mybir.AluOpType.add)

            # head: q^T/k^T for pair 0, then all of v
            wt01 = {}
            for t in (0, 4):
                wt = emit_wr_dma(t)
                for g in emit_qk_col(t, wt):
                    g()
            wt01[1] = emit_wr_dma(1)   # prefetch pair0's first filler weights
            for s in range(NSC):
                emit_v_group(s)

            # B pairs 0-2, with pair p+1's q^T/k^T production dribbled
            # into the chunk stream one matmul at a time; weight slices
            # prefetched one pair ahead
            for pair in range(3):
                t_lo, t_hi = pair + 1, pair + 5
                wt_lo = wt01.pop(t_lo, None) or emit_wr_dma(t_lo)
                fillers = list(emit_qk_col(t_lo, wt_lo, fine=True))
                wt_hi = emit_wr_dma(t_hi)
                if pair < 2:
                    wt01[pair + 2] = emit_wr_dma(pair + 2)
                fillers.extend(emit_qk_col(t_hi, wt_hi, fine=True))
                emit_B_pair(pair, fillers, psA)

        # ---- pair 3 + projection (xT/wv freed; wo loads into that space)
        with tc.tile_pool(name="wop", bufs=1) as wop, \
             tc.tile_pool(name="osb", bufs=2) as osb_pool, \
             tc.tile_pool(name="psC", bufs=2, space="PSUM") as psC:
            wo_sb = [wop.tile([P, EMB], F32R, tag=f"wo{t}", name=f"wo{t}")
                     for t in range(4)]
            for t in range(4):
                nc.sync.dma_start(wo_sb[t][:], wo[t * P:(t + 1) * P, :])

            cstate = {}

            def one_c_mm(s, y, t):
                def go():
                    if t == 0:
                        cstate[(s, y)] = psC.tile([P, 512], F32, tag="psC_t",
                                                  name=f"psc{s}_{y}")
                    ps = cstate[(s, y)]
                    nc.tensor.matmul(
                        ps[:],
                        outT[t][:, s * P:(s + 1) * P],
                        wo_sb[t][:, y * 512:(y + 1) * 512],
                        start=(t == 0), stop=(t == 3))
                    if t == 3:
                        ot = osb_pool.tile([P, 512], F32, tag="osb",
                                           name=f"osb{s}_{y}")
                        nc.vector.tensor_copy(ot[:], ps[:])
                        nc.sync.dma_start(
                            out[s * P:(s + 1) * P, y * 512:(y + 1) * 512],
                            ot[:])
                return go

            def emit_C_jb(jb):
                return [one_c_mm(s, y, t)
                        for s in range(4 * jb, 4 * jb + 4)
                        for y in range(EMB // 512)
                        for t in range(4)]

            emit_B_pair(3, [], psC, after_jb=emit_C_jb, dynamic=True, flush=True)

    nc.compile()
    return nc


def get_nc():
    global _CACHED
    if _CACHED is None:
        _CACHED = _build()
    return _CACHED


def make_in_maps(x, W_qkv, b_qkv, W_out, b_out):
    x = np.asarray(x, dtype=np.float32)
    W_qkv = np.asarray(W_qkv, dtype=np.float32)
    b_qkv = np.asarray(b_qkv, dtype=np.float32)
    W_out = np.asarray(W_out, dtype=np.float32)
    b_out = np.asarray(b_out, dtype=np.float32)

    in_maps = []
    for core in range(N_CORES):
        b, g = divmod(core, 2)
        c0 = g * 512
        wq = W_qkv[:, c0:c0 + 512] * NORM
        wk = W_qkv[:, EMB + c0:EMB + c0 + 512]
        wv_ = W_qkv[:, 2 * EMB + c0:2 * EMB + c0 + 512]
        bq = b_qkv[c0:c0 + 512] * NORM
        bk = b_qkv[EMB + c0:EMB + c0 + 512]
        bv_ = b_qkv[2 * EMB + c0:2 * EMB + c0 + 512]
        in_maps.append({
            "xT": np.ascontiguousarray(x[b].T),
            "wqk": np.ascontiguousarray(np.concatenate([wq, wk], axis=1)),
            "wv": np.ascontiguousarray(wv_),
            "wo": np.ascontiguousarray(W_out[c0:c0 + 512, :]),
            "bqk": np.ascontiguousarray(
                np.concatenate([bq, bk]).reshape(QK_COLS // P, P).T),
            "bv": bv_.reshape(1, V_COLS),
        })
    return in_maps


def kernel(x, W_qkv, b_qkv, W_out, b_out):
    nc = get_nc()
    b_out = np.asarray(b_out, dtype=np.float32)
    in_maps = make_in_maps(x, W_qkv, b_qkv, W_out, b_out)
    res = bass_utils.run_bass_kernel_spmd(nc, in_maps, core_ids=list(range(N_CORES)))
    outp = np.empty((4, SEQ, EMB), dtype=np.float32)
    for b in range(4):
        outp[b] = res.results[2 * b]["out"] + res.results[2 * b + 1]["out"] + b_out
    return outp


# revision 43
# speedup vs baseline: 1.6826x; 1.0208x over previous
"""Multi-head attention (16 heads, E=1024, seq=2048, batch=4) on 8 NeuronCores.

Sharding: core = 2*b + g  (b = batch 0..3, g = head-group 0..1, 8 heads each).
Each core computes its batch's QKV for its 8 heads, attention, and a partial
output projection (rows of W_out for its heads); host sums the two partials
per batch and adds b_out.

On-chip layout avoids all transposes:
  - host supplies x^T [1024, 2048] per core
  - q^T,k^T computed as (W^T x^T)  -> [qk_col, seq]   (lhsT = W chunk)
  - v computed naturally as x @ W_v -> [seq, v_col]   (lhsT = x^T chunk)
  - scores^T[sk, sq] = (k^T chunk)^T.T @ q^T  (lhsT = k^T slice, rhs = q^T);
    head pairs share one PSUM tile ([A sq512 | B sq512]) with the two
    64-contraction matmuls row-packed via tile_position, so one Exp
    activation covers both heads
  - softmax denominator via an appended ones-column in the PV lhsT
  - PV: out^T[d(+1), sq] = [v | 1]^T @ attn^T, accumulated over sk chunks
  - normalize: denominator row broadcast across partitions with a K=1
    matmul, then reciprocal+multiply on DVE (deferred into the next
    sq-block's chunk stream so the PE never waits on the chain)
  - proj: y[sq, :] from lhsT = out^T tiles, rhs = W_out rows for this group

Scheduling: phase B (attention) is ACT-bound on the Exp stream, so the
QKV projections for head pair p+1 and the final projection are dribbled
into pair p's / pair 3's chunk stream one matmul at a time to fill the
PE slack. All matmuls run in float32r (TF32-like, full rate at N=512)
with fp32 PSUM accumulate.
"""

import sys

sys.path.insert(0, "/opt/trn_rl_repo")

import numpy as np

import concourse.bacc as bacc
import concourse.mybir as mybir
import concourse.tile as tile
from concourse import bass_utils

P = 128
SEQ = 2048
EMB = 1024
N_HEADS_CORE = 8
D_HEAD = 64
QK_COLS = 1024          # q(512) + k(512) for this core's heads
V_COLS = 512
VA = D_HEAD + 1         # v columns per head incl. ones column
N_CORES = 8
NORM = 0.125            # 1/sqrt(64), folded into W_q/b_q on host

F32 = mybir.dt.float32
F32R = mybir.dt.float32r
AF = mybir.ActivationFunctionType

_CACHED = None


def _build():
    nc = bacc.Bacc("TRN2", target_bir_lowering=False, debug=False,
                   enable_asserts=True, num_devices=N_CORES)

    xT = nc.dram_tensor("xT", [EMB, SEQ], F32R, kind="ExternalInput").ap()
    wqk = nc.dram_tensor("wqk", [EMB, QK_COLS], F32R, kind="ExternalInput").ap()
    wv = nc.dram_tensor("wv", [EMB, V_COLS], F32R, kind="ExternalInput").ap()
    wo = nc.dram_tensor("wo", [V_COLS, EMB], F32R, kind="ExternalInput").ap()
    bqk = nc.dram_tensor("bqk", [P, QK_COLS // P], F32, kind="ExternalInput").ap()
    bv = nc.dram_tensor("bv", [1, V_COLS], F32, kind="ExternalInput").ap()
    out = nc.dram_tensor("out", [SEQ, EMB], F32, kind="ExternalOutput").ap()

    KC = EMB // P          # 8 contraction chunks
    NSC = SEQ // P         # 16 seq chunks of 128
    NJB = SEQ // 512       # 4 sq blocks of 512

    with tile.TileContext(nc) as tc:
      with tc.tile_pool(name="persist", bufs=1) as persist, \
           tc.tile_pool(name="qkT", bufs=2) as qkT_pool, \
           tc.tile_pool(name="oTp", bufs=1) as oT_pool, \
           tc.tile_pool(name="attn", bufs=2) as attn_pool, \
           tc.tile_pool(name="nrm", bufs=2) as nrm_pool, \
           tc.tile_pool(name="ps_s", bufs=2, space="PSUM") as ps_s_pool, \
           tc.tile_pool(name="ps_o0", bufs=1, space="PSUM") as ps_o0_pool, \
           tc.tile_pool(name="ps_o1", bufs=1, space="PSUM") as ps_o1_pool:
        ps_o_pools = [ps_o0_pool, ps_o1_pool]
        vsb = [persist.tile([P, N_HEADS_CORE * VA], F32R, tag=f"v{s}", name=f"v{s}")
               for s in range(NSC)]
        bqk_sb = persist.tile([P, QK_COLS // P], F32, tag="bqk")
        bv_sb = persist.tile([P, V_COLS], F32, tag="bv")
        nc.sync.dma_start(bqk_sb[:], bqk)
        nc.sync.dma_start(bv_sb[:], bv[0:1, :].broadcast_to([P, V_COLS]))
        ones_sb = persist.tile([P, D_HEAD], F32R, tag="ones")
        nc.vector.tensor_scalar(ones_sb[:], bv_sb[:, 0:D_HEAD], 0.0, 1.0,
                                mybir.AluOpType.mult, mybir.AluOpType.add)

        qT = {}
        kT = {}
        outT = [oT_pool.tile([P, SEQ], F32R, tag=f"oT{t}", name=f"oT{t}")
                for t in range(4)]

        pending = [None]

        def emit_B_pair(t, fillers, scratch_pool, after_jb=None, dynamic=False,
                        flush=False):
            """Head pair (2t, 2t+1): rows 0-63 / 64-127 of qT[t]/kT[t].
            Per chunk one ps_s [128,1024] = [A sq512 | B sq512]; scores
            row-packed, one exp for both heads, PV splits to per-head
            accumulators. `fillers` are thunks sprinkled into the chunk
            stream to fill PE slack under the ACT-bound exp pipeline."""
            kTh = kT[t]
            qTh = qT[t]
            it = 0
            fi = 0
            nfill = len(fillers)
            for j in range(NJB):
                sq0 = j * 512
                ps_os = [ps_o_pools[hh].tile([VA, 512], F32, tag=f"ps_o{hh}",
                                             name=f"ps_o{t}_{j}_{hh}")
                         for hh in range(2)]

                def scores(c):
                    ps_s = ps_s_pool.tile([P, 2 * 512], F32, tag="ps_s",
                                          name=f"ps_s{t}_{j}_{c}")
                    for hh in range(2):
                        pr = hh * D_HEAD
                        nc.tensor.matmul(
                            ps_s[:, hh * 512:(hh + 1) * 512],
                            kTh[pr:pr + D_HEAD, c * P:(c + 1) * P],
                            qTh[pr:pr + D_HEAD, sq0:sq0 + 512],
                            start=True, stop=True, tile_position=(pr, 0))
                    return ps_s

                ps_s = scores(0)
                for c in range(NSC):
                    at = attn_pool.tile([P, 2 * 512], F32R, tag="attnT",
                                        name=f"at{t}_{j}_{c}")
                    nc.scalar.activation(at[:], ps_s[:], AF.Exp)
                    if c + 1 < NSC:
                        ps_s = scores(c + 1)
                    va3 = vsb[c][:].rearrange("p (h c) -> p h c", c=VA)
                    for hh in range(2):
                        nc.tensor.matmul(
                            ps_os[hh][:],
                            va3[:, 2 * t + hh, :],
                            at[:, hh * 512:(hh + 1) * 512],
                            start=(c == 0), stop=(c == NSC - 1))
                    it += 1
                    if c == 10 and pending[0] is not None:
                        fin = pending[0]
                        pending[0] = None
                        fin()
                    if dynamic:
                        budget = 3
                        while fi < len(fillers) and budget > 0:
                            fillers[fi]()
                            fi += 1
                            budget -= 1
                    else:
                        while nfill and fi < (nfill * it) // 64 and fi < nfill:
                            fillers[fi]()
                            fi += 1

                # stage 1 (DVE): evacuate ps_o, reciprocal of denominators
                outUs = []
                rcasts = []
                for hh in range(2):
                    outU = nrm_pool.tile([VA, 512], F32, tag=f"outU{hh}",
                                         name=f"outU{t}_{j}_{hh}", bufs=1)
                    nc.vector.tensor_copy(outU[:], ps_os[hh][:])
                    rcast = nrm_pool.tile([VA, 512], F32R, tag=f"rcast{hh}",
                                          name=f"rcast{t}_{j}_{hh}", bufs=1)
                    with nc.allow_low_precision(reason="denom cast to f32r"):
                        nc.vector.tensor_copy(rcast[D_HEAD:VA, :],
                                              outU[D_HEAD:VA, :])
                    outUs.append(outU)
                    rcasts.append(rcast)

                # stage 2 (PE bcast + DVE mul): deferred into the next
                # j-block's chunk stream so the PE never waits on the
                # reciprocal chain
                def make_fin(jj, sq00, oUs, rcs):
                    def fin():
                        for hh in range(2):
                            psb = scratch_pool.tile(
                                [P, 512], F32, tag=scratch_pool.name + "_t",
                                name=f"psb{t}_{jj}_{hh}")
                            nc.tensor.matmul(psb[0:D_HEAD, :],
                                             ones_sb[D_HEAD:D_HEAD + 1, :],
                                             rcs[hh][D_HEAD:VA, :],
                                             start=True, stop=True,
                                             tile_position=(D_HEAD, 0))
                            rb = nrm_pool.tile([D_HEAD, 512], F32,
                                               tag=f"rb{hh}",
                                               name=f"rb{t}_{jj}_{hh}", bufs=1)
                            nc.vector.reciprocal(rb[:], psb[0:D_HEAD, :])
                            nc.vector.tensor_mul(
                                outT[t][hh * D_HEAD:(hh + 1) * D_HEAD,
                                        sq00:sq00 + 512],
                                oUs[hh][0:D_HEAD, :], rb[:])
                        if after_jb is not None:
                            fillers.extend(after_jb(jj))
                    return fin

                pending[0] = make_fin(j, sq0, outUs, rcasts)
            if pending[0] is not None:
                fin = pending[0]
                pending[0] = None
                fin()
            while fi < len(fillers):
                fillers[fi]()
                fi += 1

        # ---- phase A scaffolding (xT, wv, rotating weight slices) ----
        with tc.tile_pool(name="xTp", bufs=1) as xTp, \
             tc.tile_pool(name="wvp", bufs=1) as wvp, \
             tc.tile_pool(name="wrot", bufs=2) as wrot, \
             tc.tile_pool(name="psA", bufs=2, space="PSUM") as psA:
            xT_sb = [xTp.tile([P, SEQ], F32R, tag=f"xT{k}", name=f"xTs{k}")
                     for k in range(KC)]
            wv_sb = [wvp.tile([P, V_COLS], F32R, tag=f"wv{k}", name=f"wvs{k}")
                     for k in range(KC)]
            for k in range(KC):
                nc.sync.dma_start(xT_sb[k][:], xT[k * P:(k + 1) * P, :])
            for k in range(KC):
                nc.sync.dma_start(wv_sb[k][:], wv[k * P:(k + 1) * P, :])

            def emit_wr_dma(t):
                wt = []
                for k in range(KC):
                    w = wrot.tile([P, P], F32R, tag=f"wr{k}", name=f"wr{t}_{k}")
                    nc.gpsimd.dma_start(
                        w[:], wqk[k * P:(k + 1) * P, t * P:(t + 1) * P])
                    wt.append(w)
                return wt

            def emit_qk_col(t, wt, fine=False):
                """One column tile of q^T (t<4) or k^T (t>=4): 4 psum groups.
                fine=True returns one thunk per matmul (32 thunks) so a
                group can be dribbled into B's per-chunk PE slack."""
                if t < 4:
                    dst = qT[t] = qkT_pool.tile([P, SEQ], F32R, tag="qTa",
                                                name=f"qT{t}")
                else:
                    dst = kT[t - 4] = qkT_pool.tile([P, SEQ], F32R, tag="kTa",
                                                    name=f"kT{t-4}")

                state = {}

                def one_mm(j, k):
                    def go():
                        if k == 0:
                            state[j] = psA.tile([P, 512], F32, tag="psA_t",
                                                name=f"psqk{t}_{j}")
                        ps = state[j]
                        nc.tensor.matmul(
                            ps[:], wt[k][:],
                            xT_sb[k][:, j * 512:(j + 1) * 512],
                            start=(k == 0), stop=(k == KC - 1))
                        if k == KC - 1:
                            nc.vector.tensor_scalar_add(
                                dst[:, j * 512:(j + 1) * 512], ps[:],
                                bqk_sb[:, t:t + 1])
                    return go

                def one_group(j):
                    def go():
                        for k in range(KC):
                            one_mm(j, k)()
                    return go

                if fine:
                    return [one_mm(j, k) for j in range(NJB) for k in range(KC)]
                return [one_group(j) for j in range(NJB)]

            def emit_v_group(s):
                ps = psA.tile([P, V_COLS], F32, tag="psA_t", name=f"psv{s}")
                for k in range(KC):
                    nc.tensor.matmul(
                        ps[:],
                        xT_sb[k][:, s * P:(s + 1) * P],
                        wv_sb[k][:],
                        start=(k == 0), stop=(k == KC - 1))
                v3 = vsb[s][:].rearrange("p (h c) -> p h c", c=VA)
                ps3 = ps[:].rearrange("p (h c) -> p h c", c=D_HEAD)
                bv3 = bv_sb[:].rearrange("p (h c) -> p h c", c=D_HEAD)
                nc.vector.tensor_add(v3[:, :, 0:D_HEAD], ps3, bv3)
                nc.vector.tensor_scalar(
                    v3[:, :, D_HEAD], bv_sb[:, 0:N_HEADS_CORE], 0.0, 1.0,
                    mybir.AluOpType.mult, mybir.AluOpType.add)

            # head: q^T/k^T for pair 0, then all of v
            wt01 = {}
            for t in (0, 4):
                wt = emit_wr_dma(t)
                for g in emit_qk_col(t, wt):
                    g()
            wt01[1] = emit_wr_dma(1)   # prefetch pair0's first filler weights
            for s in range(NSC):
                emit_v_group(s)

            # B pairs 0-2, with pair p+1's q^T/k^T production dribbled
            # into the chunk stream one matmul at a time; weight slices
            # prefetched one pair ahead
            for pair in range(3):
                t_lo, t_hi = pair + 1, pair + 5
                wt_lo = wt01.pop(t_lo, None) or emit_wr_dma(t_lo)
                fillers = list(emit_qk_col(t_lo, wt_lo, fine=True))
                wt_hi = emit_wr_dma(t_hi)
                if pair < 2:
                    wt01[pair + 2] = emit_wr_dma(pair + 2)
                fillers.extend(emit_qk_col(t_hi, wt_hi, fine=True))
                emit_B_pair(pair, fillers, psA)

        # ---- pair 3 + projection (xT/wv freed; wo loads into that space)
        with tc.tile_pool(name="wop", bufs=1) as wop, \
             tc.tile_pool(name="osb", bufs=2) as osb_pool, \
             tc.tile_pool(name="psC", bufs=2, space="PSUM") as psC:
            wo_sb = [wop.tile([P, EMB], F32R, tag=f"wo{t}", name=f"wo{t}")
                     for t in range(4)]
            for t in range(4):
                nc.sync.dma_start(wo_sb[t][:], wo[t * P:(t + 1) * P, :])

            cstate = {}

            def one_c_mm(s, y, t):
                def go():
                    if t == 0:
                        cstate[(s, y)] = psC.tile([P, 512], F32, tag="psC_t",
                                                  name=f"psc{s}_{y}")
                    ps = cstate[(s, y)]
                    nc.tensor.matmul(
                        ps[:],
                        outT[t][:, s * P:(s + 1) * P],
                        wo_sb[t][:, y * 512:(y + 1) * 512],
                        start=(t == 0), stop=(t == 3))
                    if t == 3:
                        ot = osb_pool.tile([P, 512], F32, tag="osb",
                                           name=f"osb{s}_{y}")
                        nc.vector.tensor_copy(ot[:], ps[:])
                        nc.sync.dma_start(
                            out[s * P:(s + 1) * P, y * 512:(y + 1) * 512],
                            ot[:])
                return go

            def emit_C_jb(jb):
                return [one_c_mm(s, y, t)
                        for s in range(4 * jb, 4 * jb + 4)
                        for y in range(EMB // 512)
                        for t in range(4)]

            emit_B_pair(3, [], psC, after_jb=emit_C_jb, dynamic=True, flush=True)

    nc.compile()
    return nc


def get_nc():
    global _CACHED
    if _CACHED is None:
        _CACHED = _build()
    return _CACHED


def make_in_maps(x, W_qkv, b_qkv, W_out, b_out):
    x = np.asarray(x, dtype=np.float32)
    W_qkv = np.asarray(W_qkv, dtype=np.float32)
    b_qkv = np.asarray(b_qkv, dtype=np.float32)
    W_out = np.asarray(W_out, dtype=np.float32)
    b_out = np.asarray(b_out, dtype=np.float32)

    in_maps = []
    for core in range(N_CORES):
        b, g = divmod(core, 2)
        c0 = g * 512
        wq = W_qkv[:, c0:c0 + 512] * NORM
        wk = W_qkv[:, EMB + c0:EMB + c0 + 512]
        wv_ = W_qkv[:, 2 * EMB + c0:2 * EMB + c0 + 512]
        bq = b_qkv[c0:c0 + 512] * NORM
        bk = b_qkv[EMB + c0:EMB + c0 + 512]
        bv_ = b_qkv[2 * EMB + c0:2 * EMB + c0 + 512]
        in_maps.append({
            "xT": np.ascontiguousarray(x[b].T),
            "wqk": np.ascontiguousarray(np.concatenate([wq, wk], axis=1)),
            "wv": np.ascontiguousarray(wv_),
            "wo": np.ascontiguousarray(W_out[c0:c0 + 512, :]),
            "bqk": np.ascontiguousarray(
                np.concatenate([bq, bk]).reshape(QK_COLS // P, P).T),
            "bv": bv_.reshape(1, V_COLS),
        })
    return in_maps


def kernel(x, W_qkv, b_qkv, W_out, b_out):
    nc = get_nc()
    b_out = np.asarray(b_out, dtype=np.float32)
    in_maps = make_in_maps(x, W_qkv, b_qkv, W_out, b_out)
    res = bass_utils.run_bass_kernel_spmd(nc, in_maps, core_ids=list(range(N_CORES)))
    outp = np.empty((4, SEQ, EMB), dtype=np.float32)
    for b in range(4):
        outp[b] = res.results[2 * b]["out"] + res.results[2 * b + 1]["out"] + b_out
    return outp
